# revision 3
# baseline (speedup 1.0000x reference)
"""ClassConditionalLM log-likelihood kernel for 8 Trainium2 NeuronCores.

Math (see derivation in comments below):
  out[n] = logsumexp_j( prior'_j - S'[j,n] + corr[j,n] )
where
  S'[j,n]  = sum_l maskf[l,n] * ((z_acc+acc)[l,j] - prop[l] + log(K-1))
  corr[j,n]= sum_l [votes[l,n] == j+1] * (2*acc[l,j] + log(K-1))
  prior'_j = class_prior_j - sum_l logaddexp(prop[l], 0)

Device strategy (per core, data-parallel over instances):
  - votes^T bf16 [L=128, n] streamed in chunks.
  - per symbol v in 1..64: DVE builds mask_v = (votes^T == v) in bf16 (4x mode);
    PE accumulates block-diagonal weight matmuls into PSUM rows 0..63 (corr^T).
  - maskf = (votes^T != 0) fp32; one fp32 matmul into PSUM rows 64..127 (S'^T).
  - PE transposes 128-wide column tiles; ACT does exp (with accum-sum) and ln.
"""

import math

import numpy as np
import ml_dtypes

N, L, K = 131072, 128, 64
M = 8                    # NeuronCores
NC_N = N // M            # 16384 instances per core
F = 2048                 # instances per chunk
SUB = 512                # matmul free-dim subtile (one PSUM bank)
TPT = F // 128           # transpose tiles per chunk
LOGKM1 = math.log(K - 1)

_BASS_CACHE: dict = {}


def _build_bass(nc_n: int):
    import concourse.mybir as mybir
    from concourse.bacc import Bacc
    from concourse.tile import TileContext
    from concourse.masks import make_identity

    dt = mybir.dt
    Alu = mybir.AluOpType
    Act = mybir.ActivationFunctionType

    nchunk = nc_n // F
    assert nchunk * F == nc_n

    nc = Bacc()
    votest = nc.dram_tensor("votest", [L, nc_n], dt.bfloat16, kind="ExternalInput")
    wblk = nc.dram_tensor("wblk", [L, K * K], dt.bfloat16, kind="ExternalInput")
    stab = nc.dram_tensor("stab", [L, K], dt.float32, kind="ExternalInput")
    prior = nc.dram_tensor("prior", [K, 1], dt.float32, kind="ExternalInput")
    out = nc.dram_tensor("out", [nc_n], dt.float32, kind="ExternalOutput")

    with TileContext(nc) as tc:
        with (
            tc.tile_pool(name="const", bufs=1) as cpool,
            tc.tile_pool(name="vt", bufs=3) as vpool,
            tc.tile_pool(name="mask", bufs=4) as mpool,
            tc.tile_pool(name="work", bufs=2) as wpool,
            tc.tile_pool(name="tail", bufs=4) as tpool,
            tc.tile_pool(name="pc", bufs=1, space="PSUM") as pcpool,
            tc.tile_pool(name="pt", bufs=2, space="PSUM") as ptpool,
        ):
            ident = cpool.tile([128, 128], dt.float32, tag="ident")
            make_identity(nc, ident[:])
            wblk_sb = cpool.tile([L, K * K], dt.bfloat16, tag="wblk")
            nc.sync.dma_start(out=wblk_sb[:], in_=wblk[:, :])
            stab_sb = cpool.tile([L, K], dt.float32, tag="stab")
            nc.sync.dma_start(out=stab_sb[:], in_=stab[:, :])
            prior_sb = cpool.tile([K, 1], dt.float32, tag="prior")
            nc.sync.dma_start(out=prior_sb[:], in_=prior[:, :])

            for c in range(nchunk):
                vt = vpool.tile([L, F], dt.bfloat16, tag="vt")
                nc.sync.dma_start(out=vt[:], in_=votest[:, c * F:(c + 1) * F])

                pc = pcpool.tile([128, F], dt.float32, tag="pc")

                # S' part -> PSUM rows 64..127 (fp32 matmul)
                maskf = wpool.tile([L, F], dt.float32, tag="maskf")
                nc.vector.tensor_scalar(
                    out=maskf[:], in0=vt[:], scalar1=0.0, scalar2=None,
                    op0=Alu.not_equal,
                )
                for s in range(F // SUB):
                    nc.tensor.matmul(
                        out=pc[64:128, s * SUB:(s + 1) * SUB],
                        lhsT=stab_sb[:],
                        rhs=maskf[:, s * SUB:(s + 1) * SUB],
                        start=True, stop=True,
                    )

                # corr part -> PSUM rows 0..63, accumulated over the 64 symbols
                for v in range(1, K + 1):
                    mk = mpool.tile([L, F], dt.bfloat16, tag="mask")
                    nc.vector.tensor_scalar(
                        out=mk[:], in0=vt[:], scalar1=float(v), scalar2=None,
                        op0=Alu.is_equal,
                    )
                    for s in range(F // SUB):
                        nc.tensor.matmul(
                            out=pc[0:64, s * SUB:(s + 1) * SUB],
                            lhsT=wblk_sb[:, (v - 1) * K:v * K],
                            rhs=mk[:, s * SUB:(s + 1) * SUB],
                            start=(v == 1), stop=(v == K),
                        )

                # D^T = (corr + prior') - S'   [64, F] fp32 in SBUF
                s_sb = wpool.tile([64, F], dt.float32, tag="s_sb")
                nc.scalar.copy(out=s_sb[:], in_=pc[64:128, :])
                dT = wpool.tile([64, F], dt.float32, tag="dT")
                nc.vector.scalar_tensor_tensor(
                    out=dT[:], in0=pc[0:64, :], scalar=prior_sb[:, 0:1],
                    in1=s_sb[:], op0=Alu.add, op1=Alu.subtract,
                )

                # tail: transpose 128-column tiles, then logsumexp over j
                out_sb = tpool.tile([128, TPT], dt.float32, tag="out_sb")
                for t in range(TPT):
                    pt = ptpool.tile([128, K], dt.float32, tag="pt")
                    nc.tensor.transpose(
                        out=pt[:], in_=dT[:, t * 128:(t + 1) * 128],
                        identity=ident[0:64, 0:64],
                    )
                    mneg = tpool.tile([128, 1], dt.float32, tag="mneg")
                    nc.vector.tensor_reduce(
                        out=mneg[:], in_=pt[:], axis=mybir.AxisListType.X,
                        op=Alu.max, negate=True,
                    )
                    escr = tpool.tile([128, K], dt.float32, tag="escr")
                    ssum = tpool.tile([128, 1], dt.float32, tag="ssum")
                    nc.scalar.activation(
                        out=escr[:], in_=pt[:], func=Act.Exp,
                        bias=mneg[:, 0:1], scale=1.0, accum_out=ssum[:],
                    )
                    lns = tpool.tile([128, 1], dt.float32, tag="lns")
                    nc.scalar.activation(out=lns[:], in_=ssum[:], func=Act.Ln)
                    nc.vector.tensor_tensor(
                        out=out_sb[:, t:t + 1], in0=lns[:], in1=mneg[:],
                        op=Alu.subtract,
                    )
                oview = out[c * F:(c + 1) * F].rearrange("(t p) -> p t", p=128)
                nc.sync.dma_start(out=oview, in_=out_sb[:])
    nc.finalize()
    return nc


def _get_bass(nc_n: int):
    if nc_n not in _BASS_CACHE:
        _BASS_CACHE[nc_n] = _build_bass(nc_n)
    return _BASS_CACHE[nc_n]


def _prepare_host(votes, accuracy, propensity, class_balance):
    bf16 = ml_dtypes.bfloat16
    votes = np.asarray(votes)
    accuracy = np.asarray(accuracy, dtype=np.float32)
    propensity = np.asarray(propensity, dtype=np.float32)
    class_balance = np.asarray(class_balance, dtype=np.float32)

    # values 0..64 are exact in bf16
    votesT = np.ascontiguousarray(votes.T.astype(np.float32).astype(bf16))

    z_acc = np.logaddexp(accuracy, -accuracy)
    stab = np.ascontiguousarray(
        (z_acc + accuracy - propensity[:, None] + LOGKM1).astype(np.float32)
    )
    w = 2.0 * accuracy + LOGKM1                      # [L, K]
    wblk = np.zeros((L, K, K), np.float32)
    jj = np.arange(K)
    wblk[:, jj, jj] = w                              # block-diagonal columns
    wblk = np.ascontiguousarray(wblk.reshape(L, K * K).astype(bf16))

    zprop = np.logaddexp(propensity, 0.0)
    cb = class_balance - np.log(np.sum(np.exp(class_balance - class_balance.max()))) - class_balance.max()
    priorp = np.ascontiguousarray(
        (cb - zprop.sum()).astype(np.float32).reshape(K, 1)
    )
    return votesT, wblk, stab, priorp


def _run(votes, accuracy, propensity, class_balance, trace=False):
    from concourse.bass_utils import run_bass_kernel_spmd

    votesT, wblk, stab, priorp = _prepare_host(
        votes, accuracy, propensity, class_balance
    )
    nc = _get_bass(NC_N)
    in_maps = []
    for c in range(M):
        in_maps.append({
            "votest": np.ascontiguousarray(votesT[:, c * NC_N:(c + 1) * NC_N]),
            "wblk": wblk,
            "stab": stab,
            "prior": priorp,
        })
    res = run_bass_kernel_spmd(
        nc, in_maps, core_ids=list(range(M)), trace=trace
    )
    out = np.concatenate([r["out"] for r in res.results])
    return out.astype(np.float32), res


def kernel(votes, accuracy, propensity, class_balance):
    out, _ = _run(votes, accuracy, propensity, class_balance)
    return out


def kernel_with_stats(votes, accuracy, propensity, class_balance):
    out, res = _run(votes, accuracy, propensity, class_balance, trace=True)
    return out, res


# revision 18
# speedup vs baseline: 221.0529x; 221.0529x over previous
"""ClassConditionalLM log-likelihood kernel for 8 Trainium2 NeuronCores.

Math:
  out[n] = logsumexp_j( prior'_j - S'[j,n] + corr[j,n] )
where
  S'[j,n]  = sum_l maskf[l,n] * ((z_acc+acc)[l,j] - prop[l] + log(K-1))
  corr[j,n]= sum_l [votes[l,n] == j+1] * (2*acc[l,j] + log(K-1))
  prior'_j = class_prior_j - sum_l logaddexp(prop[l], 0)

Device strategy (per core, data-parallel over instances):
  - votes^T bf16 [L=128, n] streamed in chunks of F.
  - per symbol v in 1..64: DVE builds mask_v = (votes^T == v) in bf16 (4x mode);
    PE accumulates 32-wide block-diagonal weight matmuls into PSUM rows 0..63.
  - S' is folded into the same PSUM rows with NEGATED hi/lo bf16 weights
    (rhs = maskf), so PSUM = corr - S' directly; no separate drain needed.
  - Tail: PE transposes 128-column tiles of D^T; ACT does exp with accum-sum
    (only Exp is used during the loop so the activation table loads once).
    All ln's happen in one pass at the very end (one more table load).
"""

import math

import numpy as np
import ml_dtypes

N, L, K = 131072, 128, 64
M = 8                    # NeuronCores
NC_N = N // M            # 16384 instances per core
F = 2048                 # instances per chunk
SUB = 512                # matmul free-dim subtile (one PSUM bank)
TPT = F // 128           # transpose tiles per chunk
BLK = 32                 # corr lhsT block width (PE tile col granularity)
NPAIR = 8                # pairs with fp8 masks written directly by the DVE
ACT_PAIRS = 0            # extra pairs: bf16 masks on DVE, cast to fp8 on ACT
GPS = 10                 # bf16 symbol masks built on GPSIMD instead of DVE
LOGKM1 = math.log(K - 1)

_BASS_CACHE: dict = {}


def _build_bass(nc_n: int):
    import concourse.mybir as mybir
    from concourse.bacc import Bacc
    from concourse.tile import TileContext
    from concourse.masks import make_identity

    dt = mybir.dt
    Alu = mybir.AluOpType
    Act = mybir.ActivationFunctionType

    nchunk = nc_n // F
    assert nchunk * F == nc_n
    ncols = nchunk * TPT         # total 128-instance column tiles

    nc = Bacc()
    votest = nc.dram_tensor("votest", [L, nc_n], dt.bfloat16, kind="ExternalInput")
    wblk = nc.dram_tensor("wblk", [L, K * BLK], dt.bfloat16, kind="ExternalInput")
    wph = nc.dram_tensor("wph", [L, max(NPAIR + ACT_PAIRS, 1) * 2 * BLK], dt.float8e4,
                         kind="ExternalInput")
    wpl = nc.dram_tensor("wpl", [L, max(NPAIR + ACT_PAIRS, 1) * 2 * BLK], dt.float8e4,
                         kind="ExternalInput")
    nshi = nc.dram_tensor("nshi", [L, K], dt.bfloat16, kind="ExternalInput")
    nslo = nc.dram_tensor("nslo", [L, K], dt.bfloat16, kind="ExternalInput")
    prior = nc.dram_tensor("prior", [K, 1], dt.float32, kind="ExternalInput")
    out = nc.dram_tensor("out", [nc_n], dt.float32, kind="ExternalOutput")

    with TileContext(nc) as tc:
        with (
            tc.tile_pool(name="const", bufs=1) as cpool,
            tc.tile_pool(name="vt", bufs=3) as vpool,
            tc.tile_pool(name="mask", bufs=6) as mpool,
            tc.tile_pool(name="work", bufs=2) as wpool,
            tc.tile_pool(name="tail", bufs=6) as tpool,
            tc.tile_pool(name="pc", bufs=1, space="PSUM") as pcpool,
            tc.tile_pool(name="pt", bufs=2, space="PSUM") as ptpool,
        ):
            ident = cpool.tile([128, 128], dt.float32, tag="ident")
            make_identity(nc, ident[:])
            wblk_sb = cpool.tile([L, K * BLK], dt.bfloat16, tag="wblk")
            nc.sync.dma_start(out=wblk_sb[:], in_=wblk[:, :])
            wph_sb = cpool.tile([L, max(NPAIR + ACT_PAIRS, 1) * 2 * BLK], dt.float8e4, tag="wph")
            nc.sync.dma_start(out=wph_sb[:], in_=wph[:, :])
            wpl_sb = cpool.tile([L, max(NPAIR + ACT_PAIRS, 1) * 2 * BLK], dt.float8e4, tag="wpl")
            nc.sync.dma_start(out=wpl_sb[:], in_=wpl[:, :])
            shi_sb = cpool.tile([L, K], dt.bfloat16, tag="shi")
            nc.sync.dma_start(out=shi_sb[:], in_=nshi[:, :])
            slo_sb = cpool.tile([L, K], dt.bfloat16, tag="slo")
            nc.sync.dma_start(out=slo_sb[:], in_=nslo[:, :])
            prior_sb = cpool.tile([K, 1], dt.float32, tag="prior")
            nc.sync.dma_start(out=prior_sb[:], in_=prior[:, :])
            # per-column-tile logsumexp pieces, stashed until the end
            ssum_all = cpool.tile([128, ncols], dt.float32, tag="ssum_all")
            mneg_all = cpool.tile([128, ncols], dt.float32, tag="mneg_all")

            for c in range(nchunk):
                vt = vpool.tile([L, F], dt.bfloat16, tag="vt")
                nc.sync.dma_start(out=vt[:], in_=votest[:, c * F:(c + 1) * F])

                pc = pcpool.tile([64, F], dt.float32, tag="pc")

                # -S' into PSUM rows 0..63 (negated hi/lo bf16 weights)
                maskf = wpool.tile([L, F], dt.bfloat16, tag="maskf")
                nc.vector.tensor_scalar(
                    out=maskf[:], in0=vt[:], scalar1=0.0, scalar2=None,
                    op0=Alu.not_equal,
                )
                for s in range(F // SUB):
                    sl = slice(s * SUB, (s + 1) * SUB)
                    nc.tensor.matmul(
                        out=pc[:, sl], lhsT=shi_sb[:], rhs=maskf[:, sl],
                        start=True, stop=False, skip_group_check=True,
                    )
                    nc.tensor.matmul(
                        out=pc[:, sl], lhsT=slo_sb[:], rhs=maskf[:, sl],
                        start=False, stop=False, skip_group_check=True,
                    )

                # corr accumulated on top, in two 32-row windows.
                # First NPAIR symbol pairs go through fp8 DoubleRow matmuls
                # (256-deep contraction, 2x PE rate; hi/lo fp8 weight split
                # keeps precision better than bf16).
                for p in range(NPAIR + ACT_PAIRS):
                    v1 = 2 * p + 1
                    q = ((v1 - 1) // BLK) * BLK
                    mp = mpool.tile([L, 2 * F], dt.float8e4, tag="maskp")
                    if p < NPAIR:
                        # DVE writes the fp8 pair-mask directly
                        nc.vector.tensor_scalar(
                            out=mp[:, 0:F], in0=vt[:], scalar1=float(v1),
                            scalar2=None, op0=Alu.is_equal,
                        )
                        nc.vector.tensor_scalar(
                            out=mp[:, F:2 * F], in0=vt[:], scalar1=float(v1 + 1),
                            scalar2=None, op0=Alu.is_equal,
                        )
                    else:
                        # bf16 masks at DVE 4x rate, then one wide ACT cast
                        mpb = mpool.tile([L, 2 * F], dt.bfloat16, tag="maskpb")
                        nc.vector.tensor_scalar(
                            out=mpb[:, 0:F], in0=vt[:], scalar1=float(v1),
                            scalar2=None, op0=Alu.is_equal,
                        )
                        nc.vector.tensor_scalar(
                            out=mpb[:, F:2 * F], in0=vt[:], scalar1=float(v1 + 1),
                            scalar2=None, op0=Alu.is_equal,
                        )
                        nc.scalar.copy(out=mp[:], in_=mpb[:])
                    mp3 = mp[:].rearrange("l (i f) -> l i f", i=2)
                    for s in range(F // SUB):
                        for wsb in (wph_sb, wpl_sb):
                            nc.tensor.matmul(
                                out=pc[q:q + BLK, s * SUB:(s + 1) * SUB],
                                lhsT=wsb[:, p * 2 * BLK:(p + 1) * 2 * BLK]
                                .rearrange("l (i m) -> l i m", i=2),
                                rhs=mp3[:, :, s * SUB:(s + 1) * SUB],
                                start=False, stop=False,
                                perf_mode=mybir.MatmulPerfMode.DoubleRow,
                                skip_group_check=True,
                            )

                # remaining symbols in bf16; some masks built on the
                # (otherwise idle) GPSIMD engine to relieve the DVE.
                rest = list(range(2 * (NPAIR + ACT_PAIRS) + 1, K + 1))
                gp_every = max(1, len(rest) // max(GPS, 1))
                for i, v in enumerate(rest):
                    q = ((v - 1) // BLK) * BLK
                    mk = mpool.tile([L, F], dt.bfloat16, tag="mask")
                    on_gp = (i % gp_every == gp_every - 1) and (GPS > 0)
                    eng = nc.gpsimd if on_gp else nc.vector
                    eng.tensor_scalar(
                        out=mk[:], in0=vt[:], scalar1=float(v), scalar2=None,
                        op0=Alu.is_equal,
                    )
                    for s in range(F // SUB):
                        sl = slice(s * SUB, (s + 1) * SUB)
                        nc.tensor.matmul(
                            out=pc[q:q + BLK, sl],
                            lhsT=wblk_sb[:, (v - 1) * BLK:v * BLK],
                            rhs=mk[:, sl],
                            start=False, stop=(v == K),
                            skip_group_check=True,
                        )

                # D^T = PSUM + prior'  [64, F] fp32 in SBUF (on ACT: frees DVE)
                dT = wpool.tile([64, F], dt.float32, tag="dT")
                nc.scalar.activation(
                    out=dT[:], in_=pc[:, :], func=Act.Identity,
                    bias=prior_sb[:, 0:1], scale=1.0,
                )

                # tail: transpose 128-column tiles into one wide PSUM tile,
                # one batched max-reduce, then per-tile exp with accum-sum
                ptw = ptpool.tile([128, TPT * K], dt.float32, tag="ptw")
                for t in range(TPT):
                    nc.tensor.transpose(
                        out=ptw[:, t * K:(t + 1) * K],
                        in_=dT[:, t * 128:(t + 1) * 128],
                        identity=ident[0:64, 0:64],
                    )
                cols = slice(c * TPT, (c + 1) * TPT)
                nc.vector.tensor_reduce(
                    out=mneg_all[:, cols],
                    in_=ptw[:].rearrange("p (t k) -> p t k", k=K),
                    axis=mybir.AxisListType.X, op=Alu.max, negate=True,
                )
                for t in range(TPT):
                    col = c * TPT + t
                    escr = tpool.tile([128, K], dt.float32, tag="escr")
                    nc.scalar.activation(
                        out=escr[:], in_=ptw[:, t * K:(t + 1) * K], func=Act.Exp,
                        bias=mneg_all[:, col:col + 1], scale=1.0,
                        accum_out=ssum_all[:, col:col + 1],
                    )

            # finale: ln over all stashed sums, add back maxes, single DMA out
            lns = cpool.tile([128, ncols], dt.float32, tag="lns")
            nc.scalar.activation(out=lns[:], in_=ssum_all[:], func=Act.Ln)
            outT = cpool.tile([128, ncols], dt.float32, tag="outT")
            nc.vector.tensor_tensor(
                out=outT[:], in0=lns[:], in1=mneg_all[:], op=Alu.subtract,
            )
            oview = out[:].rearrange("(x p) -> p x", p=128)
            nc.sync.dma_start(out=oview, in_=outT[:])
    nc.finalize()
    return nc


def _get_bass(nc_n: int):
    if nc_n not in _BASS_CACHE:
        _BASS_CACHE[nc_n] = _build_bass(nc_n)
    return _BASS_CACHE[nc_n]


def _prepare_host(votes, accuracy, propensity, class_balance):
    bf16 = ml_dtypes.bfloat16
    votes = np.asarray(votes)
    accuracy = np.asarray(accuracy, dtype=np.float32)
    propensity = np.asarray(propensity, dtype=np.float32)
    class_balance = np.asarray(class_balance, dtype=np.float32)

    # values 0..64 are exact in bf16
    votesT = np.ascontiguousarray(votes.T.astype(np.float32).astype(bf16))

    z_acc = np.logaddexp(accuracy, -accuracy)
    stab = (z_acc + accuracy - propensity[:, None] + LOGKM1).astype(np.float32)
    shi = stab.astype(bf16)
    slo = (stab - shi.astype(np.float32)).astype(bf16)
    nshi = np.ascontiguousarray(-shi)       # negated: PSUM accumulates -S'
    nslo = np.ascontiguousarray(-slo)

    w = 2.0 * accuracy + LOGKM1                      # [L, K]
    wblk = np.zeros((L, K, BLK), np.float32)
    jj = np.arange(K)
    wblk[:, jj, jj % BLK] = w                        # 32-wide block columns
    wblk = np.ascontiguousarray(wblk.reshape(L, K * BLK).astype(bf16))

    # fp8 DoubleRow pair weights, hi/lo split
    f8 = ml_dtypes.float8_e4m3
    npair = max(NPAIR + ACT_PAIRS, 1)
    wph = np.zeros((L, npair, 2, BLK), np.float32)
    wpl = np.zeros((L, npair, 2, BLK), np.float32)
    w_hi = w.astype(f8).astype(np.float32)
    w_lo = (w - w_hi).astype(f8).astype(np.float32)
    for p in range(NPAIR + ACT_PAIRS):
        for i in range(2):
            j = 2 * p + i                            # target class row
            wph[:, p, i, j % BLK] = w_hi[:, j]
            wpl[:, p, i, j % BLK] = w_lo[:, j]
    wph = np.ascontiguousarray(wph.reshape(L, npair * 2 * BLK).astype(f8))
    wpl = np.ascontiguousarray(wpl.reshape(L, npair * 2 * BLK).astype(f8))

    zprop = np.logaddexp(propensity, 0.0)
    cbm = class_balance.max()
    cb = class_balance - (np.log(np.sum(np.exp(class_balance - cbm))) + cbm)
    priorp = np.ascontiguousarray(
        (cb - zprop.sum()).astype(np.float32).reshape(K, 1)
    )
    return votesT, wblk, wph, wpl, nshi, nslo, priorp


def _run(votes, accuracy, propensity, class_balance, trace=False):
    from concourse.bass_utils import run_bass_kernel_spmd

    votesT, wblk, wph, wpl, nshi, nslo, priorp = _prepare_host(
        votes, accuracy, propensity, class_balance
    )
    nc = _get_bass(NC_N)
    in_maps = []
    for c in range(M):
        in_maps.append({
            "votest": np.ascontiguousarray(votesT[:, c * NC_N:(c + 1) * NC_N]),
            "wblk": wblk,
            "wph": wph,
            "wpl": wpl,
            "nshi": nshi,
            "nslo": nslo,
            "prior": priorp,
        })
    res = run_bass_kernel_spmd(
        nc, in_maps, core_ids=list(range(M)), trace=trace
    )
    out = np.concatenate([r["out"] for r in res.results])
    return out.astype(np.float32), res


def kernel(votes, accuracy, propensity, class_balance):
    out, _ = _run(votes, accuracy, propensity, class_balance)
    return out


def kernel_with_stats(votes, accuracy, propensity, class_balance):
    try:
        out, res = _run(votes, accuracy, propensity, class_balance, trace=True)
    except (ImportError, ModuleNotFoundError):
        # no NTFF profiling hook in this environment; run without trace
        out, res = _run(votes, accuracy, propensity, class_balance, trace=False)
    return out, res


def simulate_ns() -> float:
    """Cost-model timeline estimate (ns) of one core's NEFF execution."""
    from concourse.timeline_sim import TimelineSim

    return TimelineSim(_get_bass(NC_N), trace=False).simulate()
